# revision 28
# baseline (speedup 1.0000x reference)
"""MoE-by-functional-group kernel for Trainium2 (8 NeuronCores, data-parallel).

Reference computation (B=4096, D=512, H=2048, O=512, 8 experts):
  - gate = softmax(x @ gate_w + gate_b); w = renormalized masked gate
  - 4 MLP experts:  relu(x @ w1 + b1) @ w2 + b2
  - 4 KAN experts:  einsum('big,eiog->ebo', bspline_basis(x), coeff*scaling)
  - out = sum_e w[:,e] * expert_out[e]
Returns (out, expert_mask, w) like the reference.

Sharding: batch is split 8 ways (512 rows/core); all expert parameters are
replicated so no collectives are needed. Each core computes its full output
shard; the host only reassembles (and undoes the transposed layouts).

On-core layout: activations are kept transposed ([feature, batch]) so that
every matmul has the 512-row batch as the moving free dim (N=512) and weights
as the 128x128 stationary operand, streamed from HBM in exactly the
consumption order (host pre-tiles them). The B-spline basis is evaluated with
the truncated-power identity  6*N3(t) = sum_m (-1)^m C(4,m) relu(t-m)^3
via a 4th finite difference over relu((u - j))^3, u = 2.5 x + 5.5 clamped to
11 (integer-cube cancellation is exact in fp32, so out-of-grid x gives an
exact 0 basis). The 1/6 and kan_scaling are folded into the coefficients on
the host. Matmuls run as float32r (full-rate fp32 path at free dim >= 256).
"""

import os
from contextlib import ExitStack

import numpy as np

import concourse.bass as bass
import concourse.mybir as mybir
import concourse.tile as tile
from concourse.bass_utils import run_bass_kernel_spmd
from concourse.masks import make_identity
from concourse.vector_clock import ScopedClock

N_CORES = 8
B, D, H, O = 4096, 512, 2048, 512
BS = B // N_CORES          # 512 batch rows per core
P = 128
NB = BS // P               # 4 batch tiles (gate only)
NI = D // P                # 4 input-feature tiles
NH = H // P                # 16 hidden tiles
NO = O // P                # 4 output tiles
NG = 8                     # B-spline basis functions per input feature
NE_MLP, NE_KAN = 4, 4

F32 = mybir.dt.float32
F32R = mybir.dt.float32r
AF = mybir.ActivationFunctionType
ALU = mybir.AluOpType


def _split_excess_waits(nc):
    """Walrus codegen caps sync waits at 1 per instruction (2 for
    EventSemaphore). Tile can attach more; hoist the extras onto same-engine
    NoOp carriers inserted immediately before the instruction."""
    for fn in nc.m.functions:
        for blk in fn.blocks:
            insts = blk.instructions
            if not any(
                i.sync_info is not None
                and i.sync_info.on_wait
                and len(i.sync_info.on_wait)
                > (2 if isinstance(i, mybir.InstEventSemaphore) else 1)
                for i in insts
            ):
                continue
            new = []
            for inst in insts:
                si = inst.sync_info
                cap = 2 if isinstance(inst, mybir.InstEventSemaphore) else 1
                waits = list(si.on_wait) if si is not None and si.on_wait else []
                if len(waits) > cap and inst.engine != mybir.EngineType.Unassigned:
                    for w in waits[cap:]:
                        new.append(
                            mybir.InstNoOp(
                                name=nc.get_next_instruction_name(),
                                engine=inst.engine,
                                ins=[],
                                outs=[],
                                sync_info=mybir.SyncInfo(on_wait=[w], on_update=[]),
                            )
                        )
                    inst.sync_info = mybir.SyncInfo(
                        on_wait=waits[:cap], on_update=list(si.on_update or [])
                    )
                new.append(inst)
            blk.instructions = new


class _FixedTileContext(tile.TileContext):
    """TileContext that legalizes walrus's one-wait-per-instruction cap on
    exit (see _split_excess_waits)."""

    def __exit__(self, *exc):
        ret = super().__exit__(*exc)
        if exc[0] is None:
            _split_excess_waits(self.nc)
        return ret


def _r(ap):
    """View an fp32 AP as float32r for the tensor engine."""
    return ap.bitcast(F32R)


def build_module():
    nc = bass.Bass(trn_type="TRN2", num_devices=N_CORES)

    xt = nc.dram_tensor("xt", [D, BS], F32R, kind="ExternalInput")
    mask = nc.dram_tensor("mask", [BS, 8], F32, kind="ExternalInput")
    gw = nc.dram_tensor("gate_w", [NI, P, 8], F32R, kind="ExternalInput")
    gb = nc.dram_tensor("gate_b", [1, 8], F32, kind="ExternalInput")
    w1t = nc.dram_tensor("w1t", [NE_MLP, NH, P, NI, P], F32R, kind="ExternalInput")
    b1t = nc.dram_tensor("b1t", [NE_MLP, P, NH], F32, kind="ExternalInput")
    w2t = nc.dram_tensor("w2t", [NE_MLP, NH, P, NO, P], F32R, kind="ExternalInput")
    b2t = nc.dram_tensor("b2t", [NE_MLP, P, NO], F32, kind="ExternalInput")
    cft = nc.dram_tensor("cefft", [NE_KAN, NI * NG, P, NO, P], F32R, kind="ExternalInput")
    out_t = nc.dram_tensor("out_t", [NO, P, BS], F32, kind="ExternalOutput")
    w_out = nc.dram_tensor("w_out", [BS, 8], F32, kind="ExternalOutput")

    with _FixedTileContext(nc) as tc, ExitStack() as ctx:
        persist = ctx.enter_context(tc.tile_pool(name="persist", bufs=1))
        wpool = ctx.enter_context(tc.tile_pool(name="wstream", bufs=8))
        hpool = ctx.enter_context(tc.tile_pool(name="hsb", bufs=3))
        scratch = ctx.enter_context(tc.tile_pool(name="scratch", bufs=1))
        gate_sc = ctx.enter_context(tc.tile_pool(name="gatesc", bufs=2))
        gate_ps = ctx.enter_context(tc.tile_pool(name="gateps", bufs=1, space="PSUM"))
        h_ps = ctx.enter_context(tc.tile_pool(name="hps", bufs=2, space="PSUM"))
        out_ps = ctx.enter_context(tc.tile_pool(name="ops", bufs=5, space="PSUM"))
        cpool = ctx.enter_context(tc.tile_pool(name="cpool", bufs=4))

        # ---- static inputs -------------------------------------------------
        ident = persist.tile([P, P], F32, tag="ident")
        make_identity(nc, ident[:])

        xt_sb = []
        for it in range(NI):
            t = persist.tile([P, BS], F32R, tag=f"xt{it}")
            nc.sync.dma_start(t[:], xt[it * P : (it + 1) * P, :])
            xt_sb.append(t)

        gw_sb = persist.tile([P, NI, 8], F32R, tag="gw")
        nc.sync.dma_start(gw_sb[:], gw[:].rearrange("t i e -> i t e"))
        gb_sb = persist.tile([P, 8], F32, tag="gb")
        nc.sync.dma_start(gb_sb[:], gb[:].to_broadcast((P, 8)))
        mask_sb = []
        for bt in range(NB):
            t = persist.tile([P, 8], F32, tag=f"mask{bt}")
            nc.sync.dma_start(t[:], mask[bt * P : (bt + 1) * P, :])
            mask_sb.append(t)
        b1_sb = persist.tile([P, NE_MLP, NH], F32, tag="b1")
        nc.sync.dma_start(b1_sb[:], b1t[:].rearrange("e p t -> p e t"))
        b2_sb = persist.tile([P, NE_MLP, NO], F32, tag="b2")
        nc.sync.dma_start(b2_sb[:], b2t[:].rearrange("e p t -> p e t"))
        negj = persist.tile([P, 12], F32, tag="negj")
        for j in range(12):
            nc.gpsimd.memset(negj[:, j : j + 1], float(-j))

        # ---- gate + w broadcast (emitted later, see expert loop) -----------
        wT = persist.tile([8, BS], F32, tag="wT")
        dram = ctx.enter_context(tc.tile_pool(name="dram", bufs=1, space="DRAM"))
        wscr = dram.tile([8, BS], F32, tag="wscr")
        wb = [
            persist.tile([P, BS], F32, tag=f"wb{e}", name=f"wb{e}")
            for e in range(8)
        ]
        wsb_tiles = []

        def emit_gate():
            """softmax(x@gate_w+b) * mask, renormalized; leaves w in [b,8]
            SBUF tiles (wsb) and writes the w output."""
            for bt in range(NB):
                gps = gate_ps.tile([P, 8], F32, tag="gps", name=f"gps{bt}")
                for it in range(NI):
                    nc.tensor.matmul(
                        gps[:],
                        xt_sb[it][:, bt * P : (bt + 1) * P],
                        gw_sb[:, it, :],
                        start=(it == 0),
                        stop=(it == NI - 1),
                    )
                logits = gate_sc.tile([P, 8], F32, tag="logits")
                nc.vector.tensor_add(logits[:], gps[:], gb_sb[:])
                eexp = gate_sc.tile([P, 8], F32, tag="eexp")
                sume = gate_sc.tile([P, 1], F32, tag="sume")
                nc.scalar.activation(eexp[:], logits[:], AF.Exp, accum_out=sume[:])
                rcp = gate_sc.tile([P, 1], F32, tag="rcp")
                nc.vector.reciprocal(rcp[:], sume[:])
                enorm = gate_sc.tile([P, 8], F32, tag="enorm")
                nc.vector.tensor_scalar(enorm[:], eexp[:], rcp[:], None, ALU.mult)
                maskd = gate_sc.tile([P, 8], F32, tag="maskd")
                s2 = gate_sc.tile([P, 1], F32, tag="s2")
                nc.vector.tensor_mul(maskd[:], enorm[:], mask_sb[bt][:])
                nc.vector.tensor_reduce(s2[:], maskd[:], mybir.AxisListType.X, ALU.add)
                nc.vector.tensor_scalar_add(s2[:], s2[:], 1e-9)
                r2 = gate_sc.tile([P, 1], F32, tag="r2")
                nc.vector.reciprocal(r2[:], s2[:])
                wsb = gate_sc.tile([P, 8], F32, tag=f"wsb{bt}", name=f"wsb{bt}")
                nc.vector.tensor_scalar(wsb[:], maskd[:], r2[:], None, ALU.mult)
                nc.sync.dma_start(w_out[bt * P : (bt + 1) * P, :], wsb[:])
                wsb_tiles.append(wsb)

        def emit_w_broadcast():
            """transpose w to [8,b], then broadcast each expert row to all 128
            partitions (SBUF zero-partition-step is illegal: bounce via DRAM)."""
            for bt in range(NB):
                tps = gate_ps.tile([8, P], F32, tag="gps", name=f"tps{bt}")
                nc.tensor.transpose(tps[:], wsb_tiles[bt][:], ident[:])
                nc.scalar.activation(wT[:, bt * P : (bt + 1) * P], tps[:], AF.Copy)
            nc.sync.dma_start(wscr[:], wT[:])
            for e in range(8):
                nc.sync.dma_start(wb[e][:], wscr[e : e + 1, :].to_broadcast((P, BS)))

        # ---- basis tiles (persistent) & acc --------------------------------
        bas_big = [
            persist.tile([P, NG, BS], F32R, tag=f"bas{it}", name=f"bas{it}")
            for it in range(NI)
        ]
        acc = [
            persist.tile([P, BS], F32, tag=f"acc{ot}", name=f"acc{ot}")
            for ot in range(NO)
        ]
        sqpool = ctx.enter_context(tc.tile_pool(name="sqpool", bufs=12))

        def emit_basis(it):
            """d4(relu(u-j)^3) over j: 6x the 8 basis funcs for i-tile `it`.
            relu/square on ScalarE, cube + the (batched, in-place) difference
            chain on VectorE. In-place r[j] -= r[j+1] over the whole j-range in
            one op is safe: the write cursor trails the +512-element read."""
            u = scratch.tile([P, BS], F32, tag="u")
            nc.vector.tensor_scalar(
                u[:], xt_sb[it][:].bitcast(F32), 2.5, 5.5, ALU.mult, ALU.add
            )
            nc.vector.tensor_scalar_min(u[:], u[:], 11.0)
            rb = scratch.tile([P, 12, BS], F32, tag="rbig")
            for j in range(12):
                nc.scalar.activation(
                    rb[:, j, :], u[:], AF.Relu, bias=negj[:, j : j + 1]
                )
                sq = sqpool.tile([P, BS], F32, tag="sq", name=f"sq{it}_{j}")
                nc.scalar.activation(sq[:], rb[:, j, :], AF.Square)
                nc.vector.tensor_mul(rb[:, j, :], sq[:], rb[:, j, :])
            nc.vector.tensor_sub(rb[:, 0:11, :], rb[:, 0:11, :], rb[:, 1:12, :])
            nc.vector.tensor_sub(rb[:, 0:10, :], rb[:, 0:10, :], rb[:, 1:11, :])
            nc.vector.tensor_sub(rb[:, 0:9, :], rb[:, 0:9, :], rb[:, 1:10, :])
            nc.vector.tensor_sub(bas_big[it][:], rb[:, 0:8, :], rb[:, 1:9, :])

        def combine(e, psums, bias_e=None):
            """acc[ot] += w[:,e] * (psums[ot] + bias);  e==0 initializes.
            A Scalar-engine copy drains PSUM immediately so the banks recycle
            without waiting for the (deeper) DVE queue."""
            for ot in range(NO):
                work = scratch.tile([P, BS], F32, tag=f"cmb{ot % 2}", name=f"cmb{e}_{ot}")
                # ScalarE drains PSUM (recycles the bank without waiting on the
                # deeper VectorE queue); VectorE does bias/scale/accumulate.
                cp = cpool.tile([P, BS], F32, tag="cp", name=f"cp{e}_{ot}")
                nc.scalar.activation(cp[:], psums[ot][:], AF.Copy)
                mul_in = cp
                if bias_e is not None:
                    nc.vector.tensor_scalar(
                        work[:], cp[:], b2_sb[:, bias_e, ot : ot + 1], None, ALU.add
                    )
                    mul_in = work
                dst = acc[ot] if e == 0 else work
                nc.vector.tensor_mul(dst[:], mul_in[:], wb[e][:])
                if e != 0:
                    nc.vector.tensor_add(acc[ot][:], acc[ot][:], work[:])

        # ---- MLP experts ---------------------------------------------------
        for e in range(NE_MLP):

            def mm1(ht):
                wu = wpool.tile([P, NI, P], F32R, tag="wu")
                nc.sync.dma_start(wu[:], w1t[e, ht])
                hp = h_ps.tile([P, BS], F32, tag="hp")
                for it in range(NI):
                    nc.tensor.matmul(
                        hp[:], wu[:, it, :], xt_sb[it][:],
                        start=(it == 0), stop=(it == NI - 1),
                    )
                hs = hpool.tile([P, BS], F32R, tag="hs")
                nc.scalar.activation(
                    hs[:], hp[:], AF.Relu, bias=b1_sb[:, e, ht : ht + 1]
                )
                return hs

            ops = [
                out_ps.tile([P, BS], F32, tag="ops", name=f"ops{e}_{ot}")
                for ot in range(NO)
            ]
            hs_prev = mm1(0)
            for ht in range(NH):
                hs_next = mm1(ht + 1) if ht + 1 < NH else None
                wu2 = wpool.tile([P, NO, P], F32R, tag="wu")
                nc.sync.dma_start(wu2[:], w2t[e, ht])
                for ot in range(NO):
                    nc.tensor.matmul(
                        ops[ot][:], wu2[:, ot, :], hs_prev[:],
                        start=(ht == 0), stop=(ht == NH - 1),
                    )
                hs_prev = hs_next
            if e == 0:
                # gate matmuls go here in the PE stream: late enough that the
                # softmax chain (DVE/ACT ping-pong) and the w transposes never
                # block matmul issue, early enough for combine(e0).
                emit_gate()
                emit_w_broadcast()
            combine(e, ops, bias_e=e)

        # Basis blocks emitted after the MLP experts: their ScalarE/VectorE ops
        # get LOWER priority than the MLP relus/drains (which gate the PE), and
        # the KAN matmuls' data dependencies still pull them in early enough.
        for it in range(NI):
            emit_basis(it)

        # ---- KAN experts ---------------------------------------------------
        for e in range(NE_KAN):
            kps = [
                out_ps.tile([P, BS], F32, tag="ops", name=f"kps{e}_{ot}")
                for ot in range(NO)
            ]
            for it in range(NI):
                for g in range(NG):
                    t = it * NG + g
                    cu = wpool.tile([P, NO, P], F32R, tag="wu")
                    nc.sync.dma_start(cu[:], cft[e, t])
                    for ot in range(NO):
                        nc.tensor.matmul(
                            kps[ot][:], cu[:, ot, :], bas_big[it][:, g, :],
                            start=(t == 0), stop=(t == NI * NG - 1),
                        )
            combine(NE_MLP + e, kps)

        # ---- store ---------------------------------------------------------
        for ot in range(NO):
            nc.sync.dma_start(out_t[ot], acc[ot][:])

    return nc


_NC_CACHE = []


def _get_module():
    if not _NC_CACHE:
        _NC_CACHE.append(build_module())
    return _NC_CACHE[0]


def _prep_weights(gate_w, gate_b, mlp_w1, mlp_b1, mlp_w2, mlp_b2,
                  kan_scaling, kan_coeff):
    f = np.float32
    gate_w = np.asarray(gate_w, f)
    c = np.ascontiguousarray
    gw = c(gate_w.reshape(NI, P, 8))
    gb = c(np.asarray(gate_b, f).reshape(1, 8))
    w1 = np.asarray(mlp_w1, f).reshape(NE_MLP, NI, P, NH, P)
    w1t = c(w1.transpose(0, 3, 2, 1, 4))            # [e, ht, i0, it, h0]
    b1t = c(np.asarray(mlp_b1, f).reshape(NE_MLP, NH, P).transpose(0, 2, 1))
    w2t = c(np.asarray(mlp_w2, f).reshape(NE_MLP, NH, P, NO, P))  # [e,ht,h0,ot,o0]
    b2t = c(np.asarray(mlp_b2, f).reshape(NE_MLP, NO, P).transpose(0, 2, 1))
    ceff = (np.asarray(kan_coeff, f) * np.asarray(kan_scaling, f)[..., None]) / f(6.0)
    # [e, i, o, g] -> [e, it, g, i0, ot, o0] with t = it*8+g
    ceff = ceff.reshape(NE_KAN, NI, P, NO, P, NG).transpose(0, 1, 5, 2, 3, 4)
    cft = c(ceff.reshape(NE_KAN, NI * NG, P, NO, P))
    return dict(gate_w=gw, gate_b=gb, w1t=w1t, b1t=b1t, w2t=w2t, b2t=b2t, cefft=cft)


def kernel(x, expert_mask, gate_w, gate_b, mlp_w1, mlp_b1, mlp_w2, mlp_b2,
           kan_scaling, kan_coeff):
    f = np.float32
    x = np.asarray(x, f)
    expert_mask = np.asarray(expert_mask, f)
    shared = _prep_weights(gate_w, gate_b, mlp_w1, mlp_b1, mlp_w2, mlp_b2,
                           kan_scaling, kan_coeff)
    in_maps = []
    for cidx in range(N_CORES):
        rows = slice(cidx * BS, (cidx + 1) * BS)
        in_maps.append(dict(
            xt=np.ascontiguousarray(x[rows].T),
            mask=np.ascontiguousarray(expert_mask[rows]),
            **shared,
        ))

    nc = _get_module()
    res = run_bass_kernel_spmd(nc, in_maps, core_ids=list(range(N_CORES)))

    out = np.empty((B, O), f)
    w = np.empty((B, 8), f)
    for cidx in range(N_CORES):
        rows = slice(cidx * BS, (cidx + 1) * BS)
        out[rows] = res.results[cidx]["out_t"].reshape(O, BS).T
        w[rows] = res.results[cidx]["w_out"]
    return out, expert_mask, w


# revision 32
# speedup vs baseline: 1.2584x; 1.2584x over previous
"""MoE-by-functional-group kernel for Trainium2 (8 NeuronCores, data-parallel).

Reference computation (B=4096, D=512, H=2048, O=512, 8 experts):
  - gate = softmax(x @ gate_w + gate_b); w = renormalized masked gate
  - 4 MLP experts:  relu(x @ w1 + b1) @ w2 + b2
  - 4 KAN experts:  einsum('big,eiog->ebo', bspline_basis(x), coeff*scaling)
  - out = sum_e w[:,e] * expert_out[e]
Returns (out, expert_mask, w) like the reference.

Sharding: batch is split 8 ways (512 rows/core); all expert parameters are
replicated so no collectives are needed. Each core computes its full output
shard; the host only reassembles (and undoes the transposed layouts).

On-core layout: activations are kept transposed ([feature, batch]) so that
every matmul has the 512-row batch as the moving free dim (N=512) and weights
as the 128x128 stationary operand, streamed from HBM in exactly the
consumption order (host pre-tiles them). The B-spline basis is evaluated with
the truncated-power identity  6*N3(t) = sum_m (-1)^m C(4,m) relu(t-m)^3
via a 4th finite difference over relu((u - j))^3, u = 2.5 x + 5.5 clamped to
11 (integer-cube cancellation is exact in fp32, so out-of-grid x gives an
exact 0 basis). The 1/6 and kan_scaling are folded into the coefficients on
the host. Matmuls run as float32r (full-rate fp32 path at free dim >= 256).
"""

import os
from contextlib import ExitStack

import numpy as np

import concourse.bass as bass
import concourse.mybir as mybir
import concourse.tile as tile
from concourse.bass_utils import run_bass_kernel_spmd
from concourse.masks import make_identity
from concourse.vector_clock import ScopedClock

N_CORES = 8
B, D, H, O = 4096, 512, 2048, 512
BS = B // N_CORES          # 512 batch rows per core
P = 128
NB = BS // P               # 4 batch tiles (gate only)
NI = D // P                # 4 input-feature tiles
NH = H // P                # 16 hidden tiles
NO = O // P                # 4 output tiles
NG = 8                     # B-spline basis functions per input feature
NE_MLP, NE_KAN = 4, 4

F32 = mybir.dt.float32
F32R = mybir.dt.float32r
AF = mybir.ActivationFunctionType
ALU = mybir.AluOpType


def _split_excess_waits(nc):
    """Walrus codegen caps sync waits at 1 per instruction (2 for
    EventSemaphore). Tile can attach more; hoist the extras onto same-engine
    NoOp carriers inserted immediately before the instruction."""
    for fn in nc.m.functions:
        for blk in fn.blocks:
            insts = blk.instructions
            if not any(
                i.sync_info is not None
                and i.sync_info.on_wait
                and len(i.sync_info.on_wait)
                > (2 if isinstance(i, mybir.InstEventSemaphore) else 1)
                for i in insts
            ):
                continue
            new = []
            for inst in insts:
                si = inst.sync_info
                cap = 2 if isinstance(inst, mybir.InstEventSemaphore) else 1
                waits = list(si.on_wait) if si is not None and si.on_wait else []
                if len(waits) > cap and inst.engine != mybir.EngineType.Unassigned:
                    for w in waits[cap:]:
                        new.append(
                            mybir.InstNoOp(
                                name=nc.get_next_instruction_name(),
                                engine=inst.engine,
                                ins=[],
                                outs=[],
                                sync_info=mybir.SyncInfo(on_wait=[w], on_update=[]),
                            )
                        )
                    inst.sync_info = mybir.SyncInfo(
                        on_wait=waits[:cap], on_update=list(si.on_update or [])
                    )
                new.append(inst)
            blk.instructions = new


class _FixedTileContext(tile.TileContext):
    """TileContext that legalizes walrus's one-wait-per-instruction cap on
    exit (see _split_excess_waits)."""

    def __exit__(self, *exc):
        ret = super().__exit__(*exc)
        if exc[0] is None:
            _split_excess_waits(self.nc)
        return ret


def _r(ap):
    """View an fp32 AP as float32r for the tensor engine."""
    return ap.bitcast(F32R)


def build_module():
    nc = bass.Bass(trn_type="TRN2", num_devices=N_CORES)

    xt = nc.dram_tensor("xt", [D, BS], F32R, kind="ExternalInput")
    mask = nc.dram_tensor("mask", [BS, 8], F32, kind="ExternalInput")
    gw = nc.dram_tensor("gate_w", [NI, P, 8], F32R, kind="ExternalInput")
    gb = nc.dram_tensor("gate_b", [1, 8], F32, kind="ExternalInput")
    w1t = nc.dram_tensor("w1t", [NE_MLP, NH, P, NI, P], F32R, kind="ExternalInput")
    b1t = nc.dram_tensor("b1t", [NE_MLP, P, NH], F32, kind="ExternalInput")
    w2t = nc.dram_tensor("w2t", [NE_MLP, NH, P, NO, P], F32R, kind="ExternalInput")
    b2t = nc.dram_tensor("b2t", [NE_MLP, P, NO], F32, kind="ExternalInput")
    cft = nc.dram_tensor("cefft", [NE_KAN, NI * NG, P, NO, P], F32R, kind="ExternalInput")
    out_t = nc.dram_tensor("out_t", [NO, P, BS], F32, kind="ExternalOutput")
    w_out = nc.dram_tensor("w_out", [BS, 8], F32, kind="ExternalOutput")

    with _FixedTileContext(nc) as tc, ExitStack() as ctx:
        persist = ctx.enter_context(tc.tile_pool(name="persist", bufs=1))
        wpool = ctx.enter_context(tc.tile_pool(name="wstream", bufs=8))
        hpool = ctx.enter_context(tc.tile_pool(name="hsb", bufs=3))
        scratch = ctx.enter_context(tc.tile_pool(name="scratch", bufs=1))
        gate_sc = ctx.enter_context(tc.tile_pool(name="gatesc", bufs=2))
        gate_ps = ctx.enter_context(tc.tile_pool(name="gateps", bufs=1, space="PSUM"))
        h_ps = ctx.enter_context(tc.tile_pool(name="hps", bufs=2, space="PSUM"))
        out_ps = ctx.enter_context(tc.tile_pool(name="ops", bufs=5, space="PSUM"))
        cpool = ctx.enter_context(tc.tile_pool(name="cpool", bufs=4))

        # ---- static inputs -------------------------------------------------
        ident = persist.tile([P, P], F32, tag="ident")
        make_identity(nc, ident[:])

        xt_sb = []
        for it in range(NI):
            t = persist.tile([P, BS], F32R, tag=f"xt{it}")
            nc.sync.dma_start(t[:], xt[it * P : (it + 1) * P, :])
            xt_sb.append(t)

        gw_sb = persist.tile([P, NI, 8], F32R, tag="gw")
        nc.sync.dma_start(gw_sb[:], gw[:].rearrange("t i e -> i t e"))
        gb_sb = persist.tile([P, 8], F32, tag="gb")
        nc.sync.dma_start(gb_sb[:], gb[:].to_broadcast((P, 8)))
        mask_sb = []
        for bt in range(NB):
            t = persist.tile([P, 8], F32, tag=f"mask{bt}")
            nc.sync.dma_start(t[:], mask[bt * P : (bt + 1) * P, :])
            mask_sb.append(t)
        b1_sb = persist.tile([P, NE_MLP, NH], F32, tag="b1")
        nc.sync.dma_start(b1_sb[:], b1t[:].rearrange("e p t -> p e t"))
        b2_sb = persist.tile([P, NE_MLP, NO], F32, tag="b2")
        nc.sync.dma_start(b2_sb[:], b2t[:].rearrange("e p t -> p e t"))
        negj = persist.tile([P, 12], F32, tag="negj")
        for j in range(12):
            nc.gpsimd.memset(negj[:, j : j + 1], float(-j))

        # ---- gate + w broadcast (emitted later, see expert loop) -----------
        wT = persist.tile([8, BS], F32, tag="wT")
        dram = ctx.enter_context(tc.tile_pool(name="dram", bufs=1, space="DRAM"))
        wscr = dram.tile([8, BS], F32, tag="wscr")
        wb = [
            persist.tile([P, BS], F32, tag=f"wb{e}", name=f"wb{e}")
            for e in range(8)
        ]
        wsb_tiles = []

        def emit_gate():
            """softmax(x@gate_w+b) * mask, renormalized; leaves w in [b,8]
            SBUF tiles (wsb) and writes the w output."""
            for bt in range(NB):
                gps = gate_ps.tile([P, 8], F32, tag="gps", name=f"gps{bt}")
                for it in range(NI):
                    nc.tensor.matmul(
                        gps[:],
                        xt_sb[it][:, bt * P : (bt + 1) * P],
                        gw_sb[:, it, :],
                        start=(it == 0),
                        stop=(it == NI - 1),
                    )
                logits = gate_sc.tile([P, 8], F32, tag="logits")
                nc.vector.tensor_add(logits[:], gps[:], gb_sb[:])
                eexp = gate_sc.tile([P, 8], F32, tag="eexp")
                sume = gate_sc.tile([P, 1], F32, tag="sume")
                nc.scalar.activation(eexp[:], logits[:], AF.Exp, accum_out=sume[:])
                rcp = gate_sc.tile([P, 1], F32, tag="rcp")
                nc.vector.reciprocal(rcp[:], sume[:])
                enorm = gate_sc.tile([P, 8], F32, tag="enorm")
                nc.vector.tensor_scalar(enorm[:], eexp[:], rcp[:], None, ALU.mult)
                maskd = gate_sc.tile([P, 8], F32, tag="maskd")
                s2 = gate_sc.tile([P, 1], F32, tag="s2")
                nc.vector.tensor_mul(maskd[:], enorm[:], mask_sb[bt][:])
                nc.vector.tensor_reduce(s2[:], maskd[:], mybir.AxisListType.X, ALU.add)
                nc.vector.tensor_scalar_add(s2[:], s2[:], 1e-9)
                r2 = gate_sc.tile([P, 1], F32, tag="r2")
                nc.vector.reciprocal(r2[:], s2[:])
                wsb = gate_sc.tile([P, 8], F32, tag=f"wsb{bt}", name=f"wsb{bt}")
                nc.vector.tensor_scalar(wsb[:], maskd[:], r2[:], None, ALU.mult)
                nc.sync.dma_start(w_out[bt * P : (bt + 1) * P, :], wsb[:])
                wsb_tiles.append(wsb)

        def emit_w_broadcast():
            """transpose w to [8,b], then broadcast each expert row to all 128
            partitions (SBUF zero-partition-step is illegal: bounce via DRAM)."""
            for bt in range(NB):
                tps = gate_ps.tile([8, P], F32, tag="gps", name=f"tps{bt}")
                nc.tensor.transpose(tps[:], wsb_tiles[bt][:], ident[:])
                nc.scalar.activation(wT[:, bt * P : (bt + 1) * P], tps[:], AF.Copy)
            nc.sync.dma_start(wscr[:], wT[:])
            for e in range(8):
                nc.sync.dma_start(wb[e][:], wscr[e : e + 1, :].to_broadcast((P, BS)))

        # ---- basis tiles (persistent) & acc --------------------------------
        bas_big = [
            persist.tile([P, NG, BS], F32R, tag=f"bas{it}", name=f"bas{it}")
            for it in range(NI)
        ]
        acc = [
            persist.tile([P, BS], F32, tag=f"acc{ot}", name=f"acc{ot}")
            for ot in range(NO)
        ]
        sqpool = ctx.enter_context(tc.tile_pool(name="sqpool", bufs=3))

        def emit_basis(it):
            """d4(relu(u-j)^3) over j: 6x the 8 basis funcs for i-tile `it`.
            relu/square on ScalarE, cube + the (batched, in-place) difference
            chain on VectorE. In-place r[j] -= r[j+1] over the whole j-range in
            one op is safe: the write cursor trails the +512-element read."""
            u = scratch.tile([P, BS], F32, tag="u")
            nc.vector.tensor_scalar(
                u[:], xt_sb[it][:].bitcast(F32), 2.5, 5.5, ALU.mult, ALU.add
            )
            nc.vector.tensor_scalar_min(u[:], u[:], 11.0)
            rb = scratch.tile([P, 12, BS], F32, tag="rbig")
            for j in range(12):
                nc.vector.tensor_scalar(
                    rb[:, j, :], u[:], float(j), 0.0, ALU.subtract, ALU.max
                )
                sq = sqpool.tile([P, BS], F32, tag="sq", name=f"sq{it}_{j}")
                nc.vector.tensor_mul(sq[:], rb[:, j, :], rb[:, j, :])
                nc.vector.tensor_mul(rb[:, j, :], sq[:], rb[:, j, :])
            nc.vector.tensor_sub(rb[:, 0:11, :], rb[:, 0:11, :], rb[:, 1:12, :])
            nc.vector.tensor_sub(rb[:, 0:10, :], rb[:, 0:10, :], rb[:, 1:11, :])
            nc.vector.tensor_sub(rb[:, 0:9, :], rb[:, 0:9, :], rb[:, 1:10, :])
            nc.vector.tensor_sub(bas_big[it][:], rb[:, 0:8, :], rb[:, 1:9, :])

        def combine(e, psums, bias_e=None):
            """acc[ot] += w[:,e] * (psums[ot] + bias);  e==0 initializes.
            A Scalar-engine copy drains PSUM immediately so the banks recycle
            without waiting for the (deeper) DVE queue."""
            for ot in range(NO):
                work = scratch.tile([P, BS], F32, tag=f"cmb{ot % 2}", name=f"cmb{e}_{ot}")
                # ScalarE drains PSUM (recycles the bank without waiting on the
                # deeper VectorE queue); VectorE does bias/scale/accumulate.
                cp = cpool.tile([P, BS], F32, tag="cp", name=f"cp{e}_{ot}")
                nc.scalar.activation(cp[:], psums[ot][:], AF.Copy)
                mul_in = cp
                if bias_e is not None:
                    nc.vector.tensor_scalar(
                        work[:], cp[:], b2_sb[:, bias_e, ot : ot + 1], None, ALU.add
                    )
                    mul_in = work
                dst = acc[ot] if e == 0 else work
                nc.vector.tensor_mul(dst[:], mul_in[:], wb[e][:])
                if e != 0:
                    nc.vector.tensor_add(acc[ot][:], acc[ot][:], work[:])

        # ---- MLP experts (one basis i-tile interleaved per expert) ---------
        for e in range(NE_MLP):
            emit_basis(e)

            def mm1(ht):
                wu = wpool.tile([P, NI, P], F32R, tag="wu")
                nc.sync.dma_start(wu[:], w1t[e, ht])
                hp = h_ps.tile([P, BS], F32, tag="hp")
                for it in range(NI):
                    nc.tensor.matmul(
                        hp[:], wu[:, it, :], xt_sb[it][:],
                        start=(it == 0), stop=(it == NI - 1),
                    )
                hs = hpool.tile([P, BS], F32R, tag="hs")
                nc.scalar.activation(
                    hs[:], hp[:], AF.Relu, bias=b1_sb[:, e, ht : ht + 1]
                )
                return hs

            ops = [
                out_ps.tile([P, BS], F32, tag="ops", name=f"ops{e}_{ot}")
                for ot in range(NO)
            ]
            hs_prev = mm1(0)
            for ht in range(NH):
                hs_next = mm1(ht + 1) if ht + 1 < NH else None
                wu2 = wpool.tile([P, NO, P], F32R, tag="wu")
                nc.sync.dma_start(wu2[:], w2t[e, ht])
                for ot in range(NO):
                    nc.tensor.matmul(
                        ops[ot][:], wu2[:, ot, :], hs_prev[:],
                        start=(ht == 0), stop=(ht == NH - 1),
                    )
                hs_prev = hs_next
            if e == 0:
                # gate matmuls go here in the PE stream: late enough that the
                # softmax chain (DVE/ACT ping-pong) and the w transposes never
                # block matmul issue, early enough for combine(e0).
                emit_gate()
                emit_w_broadcast()
            combine(e, ops, bias_e=e)

        # ---- KAN experts ---------------------------------------------------
        for e in range(NE_KAN):
            kps = [
                out_ps.tile([P, BS], F32, tag="ops", name=f"kps{e}_{ot}")
                for ot in range(NO)
            ]
            for it in range(NI):
                for g in range(NG):
                    t = it * NG + g
                    cu = wpool.tile([P, NO, P], F32R, tag="wu")
                    nc.sync.dma_start(cu[:], cft[e, t])
                    for ot in range(NO):
                        nc.tensor.matmul(
                            kps[ot][:], cu[:, ot, :], bas_big[it][:, g, :],
                            start=(t == 0), stop=(t == NI * NG - 1),
                        )
            combine(NE_MLP + e, kps)

        # ---- store ---------------------------------------------------------
        for ot in range(NO):
            nc.sync.dma_start(out_t[ot], acc[ot][:])

    return nc


_NC_CACHE = []


def _get_module():
    if not _NC_CACHE:
        _NC_CACHE.append(build_module())
    return _NC_CACHE[0]


def _prep_weights(gate_w, gate_b, mlp_w1, mlp_b1, mlp_w2, mlp_b2,
                  kan_scaling, kan_coeff):
    f = np.float32
    gate_w = np.asarray(gate_w, f)
    c = np.ascontiguousarray
    gw = c(gate_w.reshape(NI, P, 8))
    gb = c(np.asarray(gate_b, f).reshape(1, 8))
    w1 = np.asarray(mlp_w1, f).reshape(NE_MLP, NI, P, NH, P)
    w1t = c(w1.transpose(0, 3, 2, 1, 4))            # [e, ht, i0, it, h0]
    b1t = c(np.asarray(mlp_b1, f).reshape(NE_MLP, NH, P).transpose(0, 2, 1))
    w2t = c(np.asarray(mlp_w2, f).reshape(NE_MLP, NH, P, NO, P))  # [e,ht,h0,ot,o0]
    b2t = c(np.asarray(mlp_b2, f).reshape(NE_MLP, NO, P).transpose(0, 2, 1))
    ceff = (np.asarray(kan_coeff, f) * np.asarray(kan_scaling, f)[..., None]) / f(6.0)
    # [e, i, o, g] -> [e, it, g, i0, ot, o0] with t = it*8+g
    ceff = ceff.reshape(NE_KAN, NI, P, NO, P, NG).transpose(0, 1, 5, 2, 3, 4)
    cft = c(ceff.reshape(NE_KAN, NI * NG, P, NO, P))
    return dict(gate_w=gw, gate_b=gb, w1t=w1t, b1t=b1t, w2t=w2t, b2t=b2t, cefft=cft)


def kernel(x, expert_mask, gate_w, gate_b, mlp_w1, mlp_b1, mlp_w2, mlp_b2,
           kan_scaling, kan_coeff):
    f = np.float32
    x = np.asarray(x, f)
    expert_mask = np.asarray(expert_mask, f)
    shared = _prep_weights(gate_w, gate_b, mlp_w1, mlp_b1, mlp_w2, mlp_b2,
                           kan_scaling, kan_coeff)
    in_maps = []
    for cidx in range(N_CORES):
        rows = slice(cidx * BS, (cidx + 1) * BS)
        in_maps.append(dict(
            xt=np.ascontiguousarray(x[rows].T),
            mask=np.ascontiguousarray(expert_mask[rows]),
            **shared,
        ))

    nc = _get_module()
    res = run_bass_kernel_spmd(nc, in_maps, core_ids=list(range(N_CORES)))

    out = np.empty((B, O), f)
    w = np.empty((B, 8), f)
    for cidx in range(N_CORES):
        rows = slice(cidx * BS, (cidx + 1) * BS)
        out[rows] = res.results[cidx]["out_t"].reshape(O, BS).T
        w[rows] = res.results[cidx]["w_out"]
    return out, expert_mask, w


# revision 33
# speedup vs baseline: 1.2656x; 1.0058x over previous
"""MoE-by-functional-group kernel for Trainium2 (8 NeuronCores, data-parallel).

Reference computation (B=4096, D=512, H=2048, O=512, 8 experts):
  - gate = softmax(x @ gate_w + gate_b); w = renormalized masked gate
  - 4 MLP experts:  relu(x @ w1 + b1) @ w2 + b2
  - 4 KAN experts:  einsum('big,eiog->ebo', bspline_basis(x), coeff*scaling)
  - out = sum_e w[:,e] * expert_out[e]
Returns (out, expert_mask, w) like the reference.

Sharding: batch is split 8 ways (512 rows/core); all expert parameters are
replicated so no collectives are needed. Each core computes its full output
shard; the host only reassembles (and undoes the transposed layouts).

On-core layout: activations are kept transposed ([feature, batch]) so that
every matmul has the 512-row batch as the moving free dim (N=512) and weights
as the 128x128 stationary operand, streamed from HBM in exactly the
consumption order (host pre-tiles them). The B-spline basis is evaluated with
the truncated-power identity  6*N3(t) = sum_m (-1)^m C(4,m) relu(t-m)^3
via a 4th finite difference over relu((u - j))^3, u = 2.5 x + 5.5 clamped to
11 (integer-cube cancellation is exact in fp32, so out-of-grid x gives an
exact 0 basis). The 1/6 and kan_scaling are folded into the coefficients on
the host. Matmuls run as float32r (full-rate fp32 path at free dim >= 256).
"""

from contextlib import ExitStack

import numpy as np

import concourse.bass as bass
import concourse.mybir as mybir
import concourse.tile as tile
from concourse.bass_utils import run_bass_kernel_spmd
from concourse.masks import make_identity

N_CORES = 8
B, D, H, O = 4096, 512, 2048, 512
BS = B // N_CORES          # 512 batch rows per core
P = 128
NB = BS // P               # 4 batch tiles (gate only)
NI = D // P                # 4 input-feature tiles
NH = H // P                # 16 hidden tiles
NO = O // P                # 4 output tiles
NG = 8                     # B-spline basis functions per input feature
NE_MLP, NE_KAN = 4, 4

F32 = mybir.dt.float32
F32R = mybir.dt.float32r
AF = mybir.ActivationFunctionType
ALU = mybir.AluOpType


def _split_excess_waits(nc):
    """Walrus codegen caps sync waits at 1 per instruction (2 for
    EventSemaphore). Tile can attach more; hoist the extras onto same-engine
    NoOp carriers inserted immediately before the instruction."""
    for fn in nc.m.functions:
        for blk in fn.blocks:
            insts = blk.instructions
            if not any(
                i.sync_info is not None
                and i.sync_info.on_wait
                and len(i.sync_info.on_wait)
                > (2 if isinstance(i, mybir.InstEventSemaphore) else 1)
                for i in insts
            ):
                continue
            new = []
            for inst in insts:
                si = inst.sync_info
                cap = 2 if isinstance(inst, mybir.InstEventSemaphore) else 1
                waits = list(si.on_wait) if si is not None and si.on_wait else []
                if len(waits) > cap and inst.engine != mybir.EngineType.Unassigned:
                    for w in waits[cap:]:
                        new.append(
                            mybir.InstNoOp(
                                name=nc.get_next_instruction_name(),
                                engine=inst.engine,
                                ins=[],
                                outs=[],
                                sync_info=mybir.SyncInfo(on_wait=[w], on_update=[]),
                            )
                        )
                    inst.sync_info = mybir.SyncInfo(
                        on_wait=waits[:cap], on_update=list(si.on_update or [])
                    )
                new.append(inst)
            blk.instructions = new


class _FixedTileContext(tile.TileContext):
    """TileContext that legalizes walrus's one-wait-per-instruction cap on
    exit (see _split_excess_waits)."""

    def __exit__(self, *exc):
        ret = super().__exit__(*exc)
        if exc[0] is None:
            _split_excess_waits(self.nc)
        return ret


def build_module():
    nc = bass.Bass(trn_type="TRN2", num_devices=N_CORES)

    xt = nc.dram_tensor("xt", [D, BS], F32R, kind="ExternalInput")
    mask = nc.dram_tensor("mask", [BS, 8], F32, kind="ExternalInput")
    gw = nc.dram_tensor("gate_w", [NI, P, 8], F32R, kind="ExternalInput")
    gb = nc.dram_tensor("gate_b", [1, 8], F32, kind="ExternalInput")
    w1t = nc.dram_tensor("w1t", [NE_MLP, NH, P, NI, P], F32R, kind="ExternalInput")
    b1t = nc.dram_tensor("b1t", [NE_MLP, P, NH], F32, kind="ExternalInput")
    w2t = nc.dram_tensor("w2t", [NE_MLP, NH, P, NO, P], F32R, kind="ExternalInput")
    b2t = nc.dram_tensor("b2t", [NE_MLP, P, NO], F32, kind="ExternalInput")
    cft = nc.dram_tensor("cefft", [NE_KAN, NI * NG, P, NO, P], F32R, kind="ExternalInput")
    out_t = nc.dram_tensor("out_t", [NO, P, BS], F32, kind="ExternalOutput")
    w_out = nc.dram_tensor("w_out", [BS, 8], F32, kind="ExternalOutput")

    with _FixedTileContext(nc) as tc, ExitStack() as ctx:
        persist = ctx.enter_context(tc.tile_pool(name="persist", bufs=1))
        wpool = ctx.enter_context(tc.tile_pool(name="wstream", bufs=8))
        hpool = ctx.enter_context(tc.tile_pool(name="hsb", bufs=3))
        scratch = ctx.enter_context(tc.tile_pool(name="scratch", bufs=1))
        gate_sc = ctx.enter_context(tc.tile_pool(name="gatesc", bufs=2))
        gate_ps = ctx.enter_context(tc.tile_pool(name="gateps", bufs=1, space="PSUM"))
        h_ps = ctx.enter_context(tc.tile_pool(name="hps", bufs=2, space="PSUM"))
        out_ps = ctx.enter_context(tc.tile_pool(name="ops", bufs=5, space="PSUM"))
        cpool = ctx.enter_context(tc.tile_pool(name="cpool", bufs=4))

        # ---- static inputs -------------------------------------------------
        ident = persist.tile([P, P], F32, tag="ident")
        make_identity(nc, ident[:])

        xt_sb = []
        for it in range(NI):
            t = persist.tile([P, BS], F32R, tag=f"xt{it}")
            nc.sync.dma_start(t[:], xt[it * P : (it + 1) * P, :])
            xt_sb.append(t)

        gw_sb = persist.tile([P, NI, 8], F32R, tag="gw")
        nc.sync.dma_start(gw_sb[:], gw[:].rearrange("t i e -> i t e"))
        gb_sb = persist.tile([P, 8], F32, tag="gb")
        nc.sync.dma_start(gb_sb[:], gb[:].to_broadcast((P, 8)))
        mask_sb = []
        for bt in range(NB):
            t = persist.tile([P, 8], F32, tag=f"mask{bt}")
            nc.sync.dma_start(t[:], mask[bt * P : (bt + 1) * P, :])
            mask_sb.append(t)
        b1_sb = persist.tile([P, NE_MLP, NH], F32, tag="b1")
        nc.sync.dma_start(b1_sb[:], b1t[:].rearrange("e p t -> p e t"))
        b2_sb = persist.tile([P, NE_MLP, NO], F32, tag="b2")
        nc.sync.dma_start(b2_sb[:], b2t[:].rearrange("e p t -> p e t"))

        # ---- gate + w broadcast (emitted later, see expert loop) -----------
        wT = persist.tile([8, BS], F32, tag="wT")
        dram = ctx.enter_context(tc.tile_pool(name="dram", bufs=1, space="DRAM"))
        wscr = dram.tile([8, BS], F32, tag="wscr")
        wb = [
            persist.tile([P, BS], F32, tag=f"wb{e}", name=f"wb{e}")
            for e in range(8)
        ]
        wsb_tiles = []

        def emit_gate():
            """softmax(x@gate_w+b) * mask, renormalized; leaves w in [b,8]
            SBUF tiles (wsb) and writes the w output."""
            for bt in range(NB):
                gps = gate_ps.tile([P, 8], F32, tag="gps", name=f"gps{bt}")
                for it in range(NI):
                    nc.tensor.matmul(
                        gps[:],
                        xt_sb[it][:, bt * P : (bt + 1) * P],
                        gw_sb[:, it, :],
                        start=(it == 0),
                        stop=(it == NI - 1),
                    )
                logits = gate_sc.tile([P, 8], F32, tag="logits")
                nc.vector.tensor_add(logits[:], gps[:], gb_sb[:])
                eexp = gate_sc.tile([P, 8], F32, tag="eexp")
                sume = gate_sc.tile([P, 1], F32, tag="sume")
                nc.scalar.activation(eexp[:], logits[:], AF.Exp, accum_out=sume[:])
                rcp = gate_sc.tile([P, 1], F32, tag="rcp")
                nc.vector.reciprocal(rcp[:], sume[:])
                enorm = gate_sc.tile([P, 8], F32, tag="enorm")
                nc.vector.tensor_scalar(enorm[:], eexp[:], rcp[:], None, ALU.mult)
                maskd = gate_sc.tile([P, 8], F32, tag="maskd")
                s2 = gate_sc.tile([P, 1], F32, tag="s2")
                nc.vector.tensor_mul(maskd[:], enorm[:], mask_sb[bt][:])
                nc.vector.tensor_reduce(s2[:], maskd[:], mybir.AxisListType.X, ALU.add)
                nc.vector.tensor_scalar_add(s2[:], s2[:], 1e-9)
                r2 = gate_sc.tile([P, 1], F32, tag="r2")
                nc.vector.reciprocal(r2[:], s2[:])
                wsb = gate_sc.tile([P, 8], F32, tag=f"wsb{bt}", name=f"wsb{bt}")
                nc.vector.tensor_scalar(wsb[:], maskd[:], r2[:], None, ALU.mult)
                nc.sync.dma_start(w_out[bt * P : (bt + 1) * P, :], wsb[:])
                wsb_tiles.append(wsb)

        def emit_w_broadcast():
            """transpose w to [8,b], then broadcast each expert row to all 128
            partitions (SBUF zero-partition-step is illegal: bounce via DRAM)."""
            for bt in range(NB):
                tps = gate_ps.tile([8, P], F32, tag="gps", name=f"tps{bt}")
                nc.tensor.transpose(tps[:], wsb_tiles[bt][:], ident[:])
                nc.scalar.activation(wT[:, bt * P : (bt + 1) * P], tps[:], AF.Copy)
            nc.sync.dma_start(wscr[:], wT[:])
            for e in range(8):
                nc.sync.dma_start(wb[e][:], wscr[e : e + 1, :].to_broadcast((P, BS)))

        # ---- basis tiles (persistent) & acc --------------------------------
        bas_big = [
            persist.tile([P, NG, BS], F32R, tag=f"bas{it}", name=f"bas{it}")
            for it in range(NI)
        ]
        acc = [
            persist.tile([P, BS], F32, tag=f"acc{ot}", name=f"acc{ot}")
            for ot in range(NO)
        ]
        sqpool = ctx.enter_context(tc.tile_pool(name="sqpool", bufs=3))

        def emit_basis(it):
            """d4(relu(u-j)^3) over j: 6x the 8 basis funcs for i-tile `it`.
            relu/square on ScalarE, cube + the (batched, in-place) difference
            chain on VectorE. In-place r[j] -= r[j+1] over the whole j-range in
            one op is safe: the write cursor trails the +512-element read."""
            u = scratch.tile([P, BS], F32, tag="u")
            nc.vector.tensor_scalar(
                u[:], xt_sb[it][:].bitcast(F32), 2.5, 5.5, ALU.mult, ALU.add
            )
            nc.vector.tensor_scalar_min(u[:], u[:], 11.0)
            rb = scratch.tile([P, 12, BS], F32, tag="rbig")
            for j in range(12):
                nc.vector.tensor_scalar(
                    rb[:, j, :], u[:], float(j), 0.0, ALU.subtract, ALU.max
                )
                sq = sqpool.tile([P, BS], F32, tag="sq", name=f"sq{it}_{j}")
                nc.vector.tensor_mul(sq[:], rb[:, j, :], rb[:, j, :])
                nc.vector.tensor_mul(rb[:, j, :], sq[:], rb[:, j, :])
            nc.vector.tensor_sub(rb[:, 0:11, :], rb[:, 0:11, :], rb[:, 1:12, :])
            nc.vector.tensor_sub(rb[:, 0:10, :], rb[:, 0:10, :], rb[:, 1:11, :])
            nc.vector.tensor_sub(rb[:, 0:9, :], rb[:, 0:9, :], rb[:, 1:10, :])
            nc.vector.tensor_sub(bas_big[it][:], rb[:, 0:8, :], rb[:, 1:9, :])

        def combine(e, psums, bias_e=None):
            """acc[ot] += w[:,e] * (psums[ot] + bias);  e==0 initializes.
            A Scalar-engine copy drains PSUM immediately so the banks recycle
            without waiting for the (deeper) DVE queue."""
            for ot in range(NO):
                work = scratch.tile([P, BS], F32, tag=f"cmb{ot % 2}", name=f"cmb{e}_{ot}")
                # ScalarE drains PSUM (recycles the bank without waiting on the
                # deeper VectorE queue); VectorE does bias/scale/accumulate.
                cp = cpool.tile([P, BS], F32, tag="cp", name=f"cp{e}_{ot}")
                nc.scalar.activation(cp[:], psums[ot][:], AF.Copy)
                mul_in = cp
                if bias_e is not None:
                    nc.vector.tensor_scalar(
                        work[:], cp[:], b2_sb[:, bias_e, ot : ot + 1], None, ALU.add
                    )
                    mul_in = work
                dst = acc[ot] if e == 0 else work
                nc.vector.tensor_mul(dst[:], mul_in[:], wb[e][:])
                if e != 0:
                    nc.vector.tensor_add(acc[ot][:], acc[ot][:], work[:])

        # ---- MLP experts (one basis i-tile interleaved per expert) ---------
        for e in range(NE_MLP):
            emit_basis(e)

            def mm1(ht):
                wu = wpool.tile([P, NI, P], F32R, tag="wu")
                nc.sync.dma_start(wu[:], w1t[e, ht])
                hp = h_ps.tile([P, BS], F32, tag="hp")
                for it in range(NI):
                    nc.tensor.matmul(
                        hp[:], wu[:, it, :], xt_sb[it][:],
                        start=(it == 0), stop=(it == NI - 1),
                    )
                hs = hpool.tile([P, BS], F32R, tag="hs")
                nc.scalar.activation(
                    hs[:], hp[:], AF.Relu, bias=b1_sb[:, e, ht : ht + 1]
                )
                return hs

            ops = [
                out_ps.tile([P, BS], F32, tag="ops", name=f"ops{e}_{ot}")
                for ot in range(NO)
            ]
            hs_prev = mm1(0)
            for ht in range(NH):
                hs_next = mm1(ht + 1) if ht + 1 < NH else None
                wu2 = wpool.tile([P, NO, P], F32R, tag="wu")
                nc.sync.dma_start(wu2[:], w2t[e, ht])
                for ot in range(NO):
                    nc.tensor.matmul(
                        ops[ot][:], wu2[:, ot, :], hs_prev[:],
                        start=(ht == 0), stop=(ht == NH - 1),
                    )
                hs_prev = hs_next
            if e == 0:
                # gate matmuls go here in the PE stream: late enough that the
                # softmax chain (DVE/ACT ping-pong) and the w transposes never
                # block matmul issue, early enough for combine(e0).
                emit_gate()
                emit_w_broadcast()
            combine(e, ops, bias_e=e)

        # ---- KAN experts ---------------------------------------------------
        for e in range(NE_KAN):
            kps = [
                out_ps.tile([P, BS], F32, tag="ops", name=f"kps{e}_{ot}")
                for ot in range(NO)
            ]
            for it in range(NI):
                for g in range(NG):
                    t = it * NG + g
                    cu = wpool.tile([P, NO, P], F32R, tag="wu")
                    nc.sync.dma_start(cu[:], cft[e, t])
                    for ot in range(NO):
                        nc.tensor.matmul(
                            kps[ot][:], cu[:, ot, :], bas_big[it][:, g, :],
                            start=(t == 0), stop=(t == NI * NG - 1),
                        )
            combine(NE_MLP + e, kps)

        # ---- store ---------------------------------------------------------
        for ot in range(NO):
            nc.sync.dma_start(out_t[ot], acc[ot][:])

    return nc


_NC_CACHE = []


def _get_module():
    if not _NC_CACHE:
        _NC_CACHE.append(build_module())
    return _NC_CACHE[0]


def _prep_weights(gate_w, gate_b, mlp_w1, mlp_b1, mlp_w2, mlp_b2,
                  kan_scaling, kan_coeff):
    f = np.float32
    gate_w = np.asarray(gate_w, f)
    c = np.ascontiguousarray
    gw = c(gate_w.reshape(NI, P, 8))
    gb = c(np.asarray(gate_b, f).reshape(1, 8))
    w1 = np.asarray(mlp_w1, f).reshape(NE_MLP, NI, P, NH, P)
    w1t = c(w1.transpose(0, 3, 2, 1, 4))            # [e, ht, i0, it, h0]
    b1t = c(np.asarray(mlp_b1, f).reshape(NE_MLP, NH, P).transpose(0, 2, 1))
    w2t = c(np.asarray(mlp_w2, f).reshape(NE_MLP, NH, P, NO, P))  # [e,ht,h0,ot,o0]
    b2t = c(np.asarray(mlp_b2, f).reshape(NE_MLP, NO, P).transpose(0, 2, 1))
    ceff = (np.asarray(kan_coeff, f) * np.asarray(kan_scaling, f)[..., None]) / f(6.0)
    # [e, i, o, g] -> [e, it, g, i0, ot, o0] with t = it*8+g
    ceff = ceff.reshape(NE_KAN, NI, P, NO, P, NG).transpose(0, 1, 5, 2, 3, 4)
    cft = c(ceff.reshape(NE_KAN, NI * NG, P, NO, P))
    return dict(gate_w=gw, gate_b=gb, w1t=w1t, b1t=b1t, w2t=w2t, b2t=b2t, cefft=cft)


def kernel(x, expert_mask, gate_w, gate_b, mlp_w1, mlp_b1, mlp_w2, mlp_b2,
           kan_scaling, kan_coeff):
    f = np.float32
    x = np.asarray(x, f)
    expert_mask = np.asarray(expert_mask, f)
    shared = _prep_weights(gate_w, gate_b, mlp_w1, mlp_b1, mlp_w2, mlp_b2,
                           kan_scaling, kan_coeff)
    in_maps = []
    for cidx in range(N_CORES):
        rows = slice(cidx * BS, (cidx + 1) * BS)
        in_maps.append(dict(
            xt=np.ascontiguousarray(x[rows].T),
            mask=np.ascontiguousarray(expert_mask[rows]),
            **shared,
        ))

    nc = _get_module()
    res = run_bass_kernel_spmd(nc, in_maps, core_ids=list(range(N_CORES)))

    out = np.empty((B, O), f)
    w = np.empty((B, 8), f)
    for cidx in range(N_CORES):
        rows = slice(cidx * BS, (cidx + 1) * BS)
        out[rows] = res.results[cidx]["out_t"].reshape(O, BS).T
        w[rows] = res.results[cidx]["w_out"]
    return out, expert_mask, w


# revision 35
# speedup vs baseline: 1.3115x; 1.0362x over previous
"""MoE-by-functional-group kernel for Trainium2 (8 NeuronCores, data-parallel).

Reference computation (B=4096, D=512, H=2048, O=512, 8 experts):
  - gate = softmax(x @ gate_w + gate_b); w = renormalized masked gate
  - 4 MLP experts:  relu(x @ w1 + b1) @ w2 + b2
  - 4 KAN experts:  einsum('big,eiog->ebo', bspline_basis(x), coeff*scaling)
  - out = sum_e w[:,e] * expert_out[e]
Returns (out, expert_mask, w) like the reference.

Sharding: batch is split 8 ways (512 rows/core); all expert parameters are
replicated so no collectives are needed. Each core computes its full output
shard; the host only reassembles (and undoes the transposed layouts).

On-core layout: activations are kept transposed ([feature, batch]) so that
every matmul has the 512-row batch as the moving free dim (N=512) and weights
as the 128x128 stationary operand, streamed from HBM in exactly the
consumption order (host pre-tiles them). The B-spline basis is evaluated with
the truncated-power identity  6*N3(t) = sum_m (-1)^m C(4,m) relu(t-m)^3
via a 4th finite difference over relu((u - j))^3, u = 2.5 x + 5.5 clamped to
11 (integer-cube cancellation is exact in fp32, so out-of-grid x gives an
exact 0 basis). The 1/6 and kan_scaling are folded into the coefficients on
the host. Matmuls run as float32r (full-rate fp32 path at free dim >= 256).
"""

from contextlib import ExitStack

import numpy as np

import concourse.bass as bass
import concourse.mybir as mybir
import concourse.tile as tile
from concourse.bass_utils import run_bass_kernel_spmd
from concourse.masks import make_identity

N_CORES = 8
B, D, H, O = 4096, 512, 2048, 512
BS = B // N_CORES          # 512 batch rows per core
P = 128
NB = BS // P               # 4 batch tiles (gate only)
NI = D // P                # 4 input-feature tiles
NH = H // P                # 16 hidden tiles
NO = O // P                # 4 output tiles
NG = 8                     # B-spline basis functions per input feature
NE_MLP, NE_KAN = 4, 4

F32 = mybir.dt.float32
F32R = mybir.dt.float32r
AF = mybir.ActivationFunctionType
ALU = mybir.AluOpType


def _split_excess_waits(nc):
    """Walrus codegen caps sync waits at 1 per instruction (2 for
    EventSemaphore). Tile can attach more; hoist the extras onto same-engine
    NoOp carriers inserted immediately before the instruction."""
    for fn in nc.m.functions:
        for blk in fn.blocks:
            insts = blk.instructions
            if not any(
                i.sync_info is not None
                and i.sync_info.on_wait
                and len(i.sync_info.on_wait)
                > (2 if isinstance(i, mybir.InstEventSemaphore) else 1)
                for i in insts
            ):
                continue
            new = []
            for inst in insts:
                si = inst.sync_info
                cap = 2 if isinstance(inst, mybir.InstEventSemaphore) else 1
                waits = list(si.on_wait) if si is not None and si.on_wait else []
                if len(waits) > cap and inst.engine != mybir.EngineType.Unassigned:
                    for w in waits[cap:]:
                        new.append(
                            mybir.InstNoOp(
                                name=nc.get_next_instruction_name(),
                                engine=inst.engine,
                                ins=[],
                                outs=[],
                                sync_info=mybir.SyncInfo(on_wait=[w], on_update=[]),
                            )
                        )
                    inst.sync_info = mybir.SyncInfo(
                        on_wait=waits[:cap], on_update=list(si.on_update or [])
                    )
                new.append(inst)
            blk.instructions = new


class _FixedTileContext(tile.TileContext):
    """TileContext that legalizes walrus's one-wait-per-instruction cap on
    exit (see _split_excess_waits)."""

    def __exit__(self, *exc):
        ret = super().__exit__(*exc)
        if exc[0] is None:
            _split_excess_waits(self.nc)
        return ret


def build_module():
    nc = bass.Bass(trn_type="TRN2", num_devices=N_CORES)

    xt = nc.dram_tensor("xt", [D, BS], F32R, kind="ExternalInput")
    mask = nc.dram_tensor("mask", [BS, 8], F32, kind="ExternalInput")
    gw = nc.dram_tensor("gate_w", [NI, P, 8], F32R, kind="ExternalInput")
    gb = nc.dram_tensor("gate_b", [1, 8], F32, kind="ExternalInput")
    w1t = nc.dram_tensor("w1t", [NE_MLP, NH, P, NI, P], F32R, kind="ExternalInput")
    b1t = nc.dram_tensor("b1t", [NE_MLP, P, NH], F32, kind="ExternalInput")
    w2t = nc.dram_tensor("w2t", [NE_MLP, NH, P, NO, P], F32R, kind="ExternalInput")
    b2t = nc.dram_tensor("b2t", [NE_MLP, P, NO], F32, kind="ExternalInput")
    cft = nc.dram_tensor("cefft", [NE_KAN, NI * NG, P, NO, P], F32R, kind="ExternalInput")
    out_t = nc.dram_tensor("out_t", [NO, P, BS], F32, kind="ExternalOutput")
    w_out = nc.dram_tensor("w_out", [BS, 8], F32, kind="ExternalOutput")

    with _FixedTileContext(nc) as tc, ExitStack() as ctx:
        persist = ctx.enter_context(tc.tile_pool(name="persist", bufs=1))
        wpool = ctx.enter_context(tc.tile_pool(name="wstream", bufs=8))
        hpool = ctx.enter_context(tc.tile_pool(name="hsb", bufs=3))
        scratch = ctx.enter_context(tc.tile_pool(name="scratch", bufs=1))
        gate_sc = ctx.enter_context(tc.tile_pool(name="gatesc", bufs=2))
        gate_ps = ctx.enter_context(tc.tile_pool(name="gateps", bufs=1, space="PSUM"))
        h_ps = ctx.enter_context(tc.tile_pool(name="hps", bufs=2, space="PSUM"))
        out_ps = ctx.enter_context(tc.tile_pool(name="ops", bufs=5, space="PSUM"))
        cpool = ctx.enter_context(tc.tile_pool(name="cpool", bufs=4))

        # ---- static inputs -------------------------------------------------
        ident = persist.tile([P, P], F32, tag="ident")
        make_identity(nc, ident[:])

        xt_sb = []
        for it in range(NI):
            t = persist.tile([P, BS], F32R, tag=f"xt{it}")
            nc.sync.dma_start(t[:], xt[it * P : (it + 1) * P, :])
            xt_sb.append(t)

        gw_sb = persist.tile([P, NI, 8], F32R, tag="gw")
        nc.sync.dma_start(gw_sb[:], gw[:].rearrange("t i e -> i t e"))
        gb_sb = persist.tile([P, 8], F32, tag="gb")
        nc.sync.dma_start(gb_sb[:], gb[:].to_broadcast((P, 8)))
        mask_sb = []
        for bt in range(NB):
            t = persist.tile([P, 8], F32, tag=f"mask{bt}")
            nc.sync.dma_start(t[:], mask[bt * P : (bt + 1) * P, :])
            mask_sb.append(t)
        b1_sb = persist.tile([P, NE_MLP, NH], F32, tag="b1")
        nc.sync.dma_start(b1_sb[:], b1t[:].rearrange("e p t -> p e t"))
        b2_sb = persist.tile([P, NE_MLP, NO], F32, tag="b2")
        nc.sync.dma_start(b2_sb[:], b2t[:].rearrange("e p t -> p e t"))

        # ---- gate + w broadcast (emitted later, see expert loop) -----------
        wT = persist.tile([8, BS], F32, tag="wT")
        dram = ctx.enter_context(tc.tile_pool(name="dram", bufs=1, space="DRAM"))
        wscr = dram.tile([8, BS], F32, tag="wscr")
        wb = [
            persist.tile([P, BS], F32, tag=f"wb{e}", name=f"wb{e}")
            for e in range(8)
        ]
        wsb_tiles = []

        def emit_gate():
            """softmax(x@gate_w+b) * mask, renormalized; leaves w in [b,8]
            SBUF tiles (wsb) and writes the w output."""
            for bt in range(NB):
                gps = gate_ps.tile([P, 8], F32, tag="gps", name=f"gps{bt}")
                for it in range(NI):
                    nc.tensor.matmul(
                        gps[:],
                        xt_sb[it][:, bt * P : (bt + 1) * P],
                        gw_sb[:, it, :],
                        start=(it == 0),
                        stop=(it == NI - 1),
                    )
                logits = gate_sc.tile([P, 8], F32, tag="logits")
                nc.vector.tensor_add(logits[:], gps[:], gb_sb[:])
                eexp = gate_sc.tile([P, 8], F32, tag="eexp")
                sume = gate_sc.tile([P, 1], F32, tag="sume")
                nc.scalar.activation(eexp[:], logits[:], AF.Exp, accum_out=sume[:])
                rcp = gate_sc.tile([P, 1], F32, tag="rcp")
                nc.vector.reciprocal(rcp[:], sume[:])
                enorm = gate_sc.tile([P, 8], F32, tag="enorm")
                nc.vector.tensor_scalar(enorm[:], eexp[:], rcp[:], None, ALU.mult)
                maskd = gate_sc.tile([P, 8], F32, tag="maskd")
                s2 = gate_sc.tile([P, 1], F32, tag="s2")
                nc.vector.tensor_mul(maskd[:], enorm[:], mask_sb[bt][:])
                nc.vector.tensor_reduce(s2[:], maskd[:], mybir.AxisListType.X, ALU.add)
                nc.vector.tensor_scalar_add(s2[:], s2[:], 1e-9)
                r2 = gate_sc.tile([P, 1], F32, tag="r2")
                nc.vector.reciprocal(r2[:], s2[:])
                wsb = gate_sc.tile([P, 8], F32, tag=f"wsb{bt}", name=f"wsb{bt}")
                nc.vector.tensor_scalar(wsb[:], maskd[:], r2[:], None, ALU.mult)
                nc.sync.dma_start(w_out[bt * P : (bt + 1) * P, :], wsb[:])
                wsb_tiles.append(wsb)

        def emit_w_broadcast():
            """transpose w to [8,b], then broadcast each expert row to all 128
            partitions (SBUF zero-partition-step is illegal: bounce via DRAM)."""
            for bt in range(NB):
                tps = gate_ps.tile([8, P], F32, tag="gps", name=f"tps{bt}")
                nc.tensor.transpose(tps[:], wsb_tiles[bt][:], ident[:])
                nc.scalar.activation(wT[:, bt * P : (bt + 1) * P], tps[:], AF.Copy)
            nc.sync.dma_start(wscr[:], wT[:])
            for e in range(8):
                nc.sync.dma_start(wb[e][:], wscr[e : e + 1, :].to_broadcast((P, BS)))

        # ---- basis tiles (persistent) & acc --------------------------------
        bas_big = [
            persist.tile([P, NG, BS], F32R, tag=f"bas{it}", name=f"bas{it}")
            for it in range(NI)
        ]
        acc = [
            persist.tile([P, BS], F32, tag=f"acc{ot}", name=f"acc{ot}")
            for ot in range(NO)
        ]
        sqpool = ctx.enter_context(tc.tile_pool(name="sqpool", bufs=3))

        def emit_basis(it):
            """d4(relu(u-j)^3) over j: 6x the 8 basis funcs for i-tile `it`.
            relu/square on ScalarE, cube + the (batched, in-place) difference
            chain on VectorE. In-place r[j] -= r[j+1] over the whole j-range in
            one op is safe: the write cursor trails the +512-element read."""
            u = scratch.tile([P, BS], F32, tag="u")
            nc.vector.tensor_scalar(
                u[:], xt_sb[it][:].bitcast(F32), 2.5, 5.5, ALU.mult, ALU.add
            )
            nc.vector.tensor_scalar_min(u[:], u[:], 11.0)
            rb = scratch.tile([P, 12, BS], F32, tag="rbig")
            # three single-engine runs (relu:DVE 2x, square:ACT, cube:DVE) so
            # neither queue ping-pongs: ACT squares run while DVE finishes the
            # relus, cubes chase the squares with sqpool slack.
            for j in range(12):
                nc.vector.tensor_scalar(
                    rb[:, j, :], u[:], float(j), 0.0, ALU.subtract, ALU.max
                )
            sqs = []
            for j in range(12):
                sq = sqpool.tile([P, BS], F32, tag="sq", name=f"sq{it}_{j}")
                nc.scalar.activation(sq[:], rb[:, j, :], AF.Square)
                sqs.append(sq)
            for j in range(12):
                nc.vector.tensor_mul(rb[:, j, :], sqs[j][:], rb[:, j, :])
            nc.vector.tensor_sub(rb[:, 0:11, :], rb[:, 0:11, :], rb[:, 1:12, :])
            nc.vector.tensor_sub(rb[:, 0:10, :], rb[:, 0:10, :], rb[:, 1:11, :])
            nc.vector.tensor_sub(rb[:, 0:9, :], rb[:, 0:9, :], rb[:, 1:10, :])
            nc.vector.tensor_sub(bas_big[it][:], rb[:, 0:8, :], rb[:, 1:9, :])

        def combine(e, psums, bias_e=None):
            """acc[ot] += w[:,e] * (psums[ot] + bias);  e==0 initializes.
            A Scalar-engine copy drains PSUM immediately so the banks recycle
            without waiting for the (deeper) DVE queue."""
            for ot in range(NO):
                work = scratch.tile([P, BS], F32, tag=f"cmb{ot % 2}", name=f"cmb{e}_{ot}")
                # ScalarE drains PSUM (recycles the bank without waiting on the
                # deeper VectorE queue); VectorE does bias/scale/accumulate.
                cp = cpool.tile([P, BS], F32, tag="cp", name=f"cp{e}_{ot}")
                nc.scalar.activation(cp[:], psums[ot][:], AF.Copy)
                mul_in = cp
                if bias_e is not None:
                    nc.vector.tensor_scalar(
                        work[:], cp[:], b2_sb[:, bias_e, ot : ot + 1], None, ALU.add
                    )
                    mul_in = work
                dst = acc[ot] if e == 0 else work
                nc.vector.tensor_mul(dst[:], mul_in[:], wb[e][:])
                if e != 0:
                    nc.vector.tensor_add(acc[ot][:], acc[ot][:], work[:])

        # ---- MLP experts (basis i-tiles interleaved, front-loaded by one
        # expert so the last block's chain clears VectorE before the KAN
        # matmuls need it) ---------------------------------------------------
        basis_at = {0: [0, 1], 1: [2], 2: [3], 3: []}
        for e in range(NE_MLP):
            for it in basis_at[e]:
                emit_basis(it)

            def mm1(ht):
                wu = wpool.tile([P, NI, P], F32R, tag="wu")
                nc.sync.dma_start(wu[:], w1t[e, ht])
                hp = h_ps.tile([P, BS], F32, tag="hp")
                for it in range(NI):
                    nc.tensor.matmul(
                        hp[:], wu[:, it, :], xt_sb[it][:],
                        start=(it == 0), stop=(it == NI - 1),
                    )
                hs = hpool.tile([P, BS], F32R, tag="hs")
                nc.scalar.activation(
                    hs[:], hp[:], AF.Relu, bias=b1_sb[:, e, ht : ht + 1]
                )
                return hs

            ops = [
                out_ps.tile([P, BS], F32, tag="ops", name=f"ops{e}_{ot}")
                for ot in range(NO)
            ]
            hs_prev = mm1(0)
            for ht in range(NH):
                hs_next = mm1(ht + 1) if ht + 1 < NH else None
                wu2 = wpool.tile([P, NO, P], F32R, tag="wu")
                nc.sync.dma_start(wu2[:], w2t[e, ht])
                for ot in range(NO):
                    nc.tensor.matmul(
                        ops[ot][:], wu2[:, ot, :], hs_prev[:],
                        start=(ht == 0), stop=(ht == NH - 1),
                    )
                hs_prev = hs_next
            if e == 0:
                # gate matmuls go here in the PE stream: late enough that the
                # softmax chain (DVE/ACT ping-pong) and the w transposes never
                # block matmul issue, early enough for combine(e0).
                emit_gate()
                emit_w_broadcast()
            combine(e, ops, bias_e=e)

        # ---- KAN experts ---------------------------------------------------
        for e in range(NE_KAN):
            kps = [
                out_ps.tile([P, BS], F32, tag="ops", name=f"kps{e}_{ot}")
                for ot in range(NO)
            ]
            for it in range(NI):
                for g in range(NG):
                    t = it * NG + g
                    cu = wpool.tile([P, NO, P], F32R, tag="wu")
                    nc.sync.dma_start(cu[:], cft[e, t])
                    for ot in range(NO):
                        nc.tensor.matmul(
                            kps[ot][:], cu[:, ot, :], bas_big[it][:, g, :],
                            start=(t == 0), stop=(t == NI * NG - 1),
                        )
            combine(NE_MLP + e, kps)

        # ---- store ---------------------------------------------------------
        for ot in range(NO):
            nc.sync.dma_start(out_t[ot], acc[ot][:])

    return nc


_NC_CACHE = []


def _get_module():
    if not _NC_CACHE:
        _NC_CACHE.append(build_module())
    return _NC_CACHE[0]


def _prep_weights(gate_w, gate_b, mlp_w1, mlp_b1, mlp_w2, mlp_b2,
                  kan_scaling, kan_coeff):
    f = np.float32
    gate_w = np.asarray(gate_w, f)
    c = np.ascontiguousarray
    gw = c(gate_w.reshape(NI, P, 8))
    gb = c(np.asarray(gate_b, f).reshape(1, 8))
    w1 = np.asarray(mlp_w1, f).reshape(NE_MLP, NI, P, NH, P)
    w1t = c(w1.transpose(0, 3, 2, 1, 4))            # [e, ht, i0, it, h0]
    b1t = c(np.asarray(mlp_b1, f).reshape(NE_MLP, NH, P).transpose(0, 2, 1))
    w2t = c(np.asarray(mlp_w2, f).reshape(NE_MLP, NH, P, NO, P))  # [e,ht,h0,ot,o0]
    b2t = c(np.asarray(mlp_b2, f).reshape(NE_MLP, NO, P).transpose(0, 2, 1))
    ceff = (np.asarray(kan_coeff, f) * np.asarray(kan_scaling, f)[..., None]) / f(6.0)
    # [e, i, o, g] -> [e, it, g, i0, ot, o0] with t = it*8+g
    ceff = ceff.reshape(NE_KAN, NI, P, NO, P, NG).transpose(0, 1, 5, 2, 3, 4)
    cft = c(ceff.reshape(NE_KAN, NI * NG, P, NO, P))
    return dict(gate_w=gw, gate_b=gb, w1t=w1t, b1t=b1t, w2t=w2t, b2t=b2t, cefft=cft)


def kernel(x, expert_mask, gate_w, gate_b, mlp_w1, mlp_b1, mlp_w2, mlp_b2,
           kan_scaling, kan_coeff):
    f = np.float32
    x = np.asarray(x, f)
    expert_mask = np.asarray(expert_mask, f)
    shared = _prep_weights(gate_w, gate_b, mlp_w1, mlp_b1, mlp_w2, mlp_b2,
                           kan_scaling, kan_coeff)
    in_maps = []
    for cidx in range(N_CORES):
        rows = slice(cidx * BS, (cidx + 1) * BS)
        in_maps.append(dict(
            xt=np.ascontiguousarray(x[rows].T),
            mask=np.ascontiguousarray(expert_mask[rows]),
            **shared,
        ))

    nc = _get_module()
    res = run_bass_kernel_spmd(nc, in_maps, core_ids=list(range(N_CORES)))

    out = np.empty((B, O), f)
    w = np.empty((B, 8), f)
    for cidx in range(N_CORES):
        rows = slice(cidx * BS, (cidx + 1) * BS)
        out[rows] = res.results[cidx]["out_t"].reshape(O, BS).T
        w[rows] = res.results[cidx]["w_out"]
    return out, expert_mask, w
